# revision 58
# baseline (speedup 1.0000x reference)
"""
Trainium2 Bass kernel for CondConv mask head (CondInst-style dynamic mask head).

Computation (fixed problem size):
  mask_feats (2, 8, 136, 200), 128 instances with per-instance 169 params
  -> per-instance 3-layer 1x1 convs over [rel_coords(2); feats(8)] -> (128,1,136,200)
  -> aligned_bilinear x2 upsample -> sigmoid -> (128, 1, 272, 400)

Strategy (8 NeuronCores, 16 instances per core):
  * Host folds rel-coordinate channels into a shared 19-row spatial matrix Z
    and a per-(instance,outchan) lhsT A0T (19, 128); layer-1 lhsT is block
    diagonal (128, 128).  All matmuls are float32r (full rate at >=256 cols).
  * Image processed in four W-phases (50 cols each, host-reordered).
    Layers 0/1 stream in 512-col units (shared PSUM pool, 6 bufs, deep
    software pipeline); y1 lands in a persistent per-phase SBUF buffer.
  * Layer 2 writes a 128-partition PSUM layout directly: partition 16b+j =
    (row-block b, instance j).  Eight accumulating matmuls per PSUM chunk,
    each with a zero-masked [128,128] lhsT (w2 of instance j at columns
    16b..16b+16, zeros elsewhere) and rhs = y1 columns of row-block b.  The
    PE computes all 128 output partitions per streamed column for free, so
    this costs no extra PE time, and the layer-2 eviction becomes a cheap
    [128, ~400] copy instead of a [16, 27200] crawl.  b2 rides in the
    sigmoid's bias operand.  17*50 <= 1024 keeps a phase in 2 PSUM banks.
  * The blocked layout needs no re-partition DMAs: each partition already
    holds its 17 rows + 1 halo slot.  Halos are filled by two tiny SBUF
    partition-shift DMAs per phase (dup row 0 for block 0; shift last rows
    down 16 partitions for blocks 1..7).
  * aligned_bilinear(t,2) per axis: out[0]=in[0]; out[2k+1]=in[k];
    out[2k]=avg(in[k-1],in[k]).  Wout rows stored 2x, row sums 4x; sigmoid
    folds the 0.5/0.25 factor into its scale operand and b2 into bias.
    Upsample staging (y2blk, fx) is fp16: halves SBUF traffic and makes the
    row-sum pass DVE 2x-mode eligible; sigmoid reads fp16, writes a
    separate fp32 tile that feeds the phase-major output DMA (the
    phase-major DRAM layout keeps DMA descriptor elements large).
  * Elementwise ops are placed by a greedy cost-model balancer with
    structural constraints: y1 evictions and upsample passes stay on
    ACT/DVE (whose scheduler cost model is accurate), GPSIMD only gets y0
    evictions whose consumers are LAG units away.
"""

import os
import numpy as np

ISOLATE = bool(int(os.environ.get('ISOLATE_CONV', '0')))
EVICT_ENGINES = tuple(os.environ.get('EVICT_ENGINES', 'act,dve').split(','))

CH = 8
CIN = 8
N_IMG, H, W = 2, 136, 200
HW = H * W                      # 27200
N_INST = 128
N_CORES = 8
IPC = 16                        # instances per core
FACTOR = 2
OH, OW = H * FACTOR, W * FACTOR  # 272, 400
BLK = 8                         # row-blocks per instance
RPB = H // BLK                  # 17 rows per block
ORPB = RPB * FACTOR             # 34 out-rows per block
K0 = 3 + N_IMG * CIN            # 19 contraction rows for layer 0

PHW = [72, 64, 64]              # W-phase widths
NPH = len(PHW)
PHOFF = [0, 72, 136]
# layer-2 chunk lengths (block-local, each fits a PSUM bank, >=256)
ML = {72: [408, 408, 408], 64: [384, 384, 320]}
# z-chunk widths per phase (starter + rest), 512-grid aligned boundaries
PHCHUNKS = [[512, 4608, 4672], [512, 4096, 4096], [512, 4096, 4096]]

UW = 512                        # layer-0/1 stream unit width
LAG = 3                         # software-pipeline distance mm0(i) | mm1(i-LAG)
M2SLACK = 3                     # units of extra lag for layer-2 matmul emission
DELAY = 9                       # units of lag before a phase's upsample bands
WARMUP = 4                      # dummy matmuls to ramp the PE p-state
PREFETCH = 5                    # units early to issue next phase's z DMAs

LAST_EXEC_TIME_NS = None
_CACHE = {}


def _units(spatial):
    """Units (off, w) of <=512 cols covering `spatial`; no sub-256 tails."""
    units = []
    off = 0
    while spatial - off > UW + 256:
        units.append((off, UW))
        off += UW
    rem = spatial - off
    if rem <= UW:
        units.append((off, rem))
    else:
        units.append((off, rem - rem // 2))
        units.append((off + rem - rem // 2, rem // 2))
    return units


def _build_program():
    import concourse.bass as bass
    import concourse.bacc as bacc
    import concourse.tile as tile
    from concourse import mybir
    from contextlib import ExitStack

    f32 = mybir.dt.float32
    f16 = mybir.dt.float16
    f32r = mybir.dt.float32r
    Alu = mybir.AluOpType
    Act = mybir.ActivationFunctionType

    nc = bacc.Bacc("TRN2", target_bir_lowering=False, debug=False)

    zd = nc.dram_tensor("z_in", [K0, HW], f32r, kind="ExternalInput").ap()
    a0d = nc.dram_tensor("a0t_in", [K0, 128], f32r, kind="ExternalInput").ap()
    wpd = nc.dram_tensor("wpack_in", [128, 130], f32r, kind="ExternalInput").ap()
    w2vd = nc.dram_tensor("w2v_in", [128, 8 * 128], f32r,
                          kind="ExternalInput").ap()
    outd = nc.dram_tensor("out", [BLK, IPC, ORPB, OW], f32,
                          kind="ExternalOutput").ap()

    with tile.TileContext(nc) as tc, ExitStack() as ctx:
        consts = ctx.enter_context(tc.tile_pool(name="consts", bufs=1))
        a0t = consts.tile([K0, 128], f32r)
        wp = consts.tile([128, 130], f32r)
        w2v = consts.tile([128, 8 * 128], f32r)
        wsc = consts.tile([128, 512], f32)      # warmup scratch
        nc.sync.dma_start(a0t[:], a0d)
        a0r = a0t[:]
        w1r = wp[:, 0:128]
        b1ap = wp[:, 128:129].bitcast(f32)
        b2ap = wp[:, 129:130].bitcast(f32)

        zs = ctx.enter_context(tc.tile_pool(name="zs", bufs=2))
        zc = ctx.enter_context(tc.tile_pool(name="zc", bufs=2))
        y0p = ctx.enter_context(tc.tile_pool(name="y0p", bufs=5))
        y1p = ctx.enter_context(tc.tile_pool(name="y1p", bufs=2))
        p01 = ctx.enter_context(tc.tile_pool(name="p01", bufs=5, space="PSUM"))
        p2 = ctx.enter_context(tc.tile_pool(name="p2", bufs=3, space="PSUM"))
        y2p = ctx.enter_context(tc.tile_pool(name="y2p", bufs=2))
        fxp = ctx.enter_context(tc.tile_pool(name="fxp", bufs=2))

        # ---- greedy elementwise engine balancer (cost-model based) ----
        busy = {"act": 0.0, "dve": 0.0, "gp": 0.0}

        def cost(e, kind, nel, psum):
            if e == "act":
                return nel * 0.8333 + 185.0
            if e == "dve":
                mult = 0.5 if kind == "add2x" else 1.0
                return nel * 1.0417 * mult + (125.0 if psum else 60.0)
            eff = 0.42 if kind.startswith("add") else 0.6
            return 95.0 + nel * 0.8333 / eff

        def pick(kind, nel, psum=False, allowed=("act", "dve", "gp")):
            best, bc = None, None
            for e in allowed:
                c = busy[e] + cost(e, kind, nel, psum)
                if bc is None or c < bc:
                    best, bc = e, c
            busy[best] += cost(best, kind, nel, psum)
            return best

        def ew_relu(dst, src, nel):
            e = pick("ts", nel, psum=True, allowed=EVICT_ENGINES)
            if e == "act":
                nc.scalar.activation(dst, src, Act.Relu)
            elif e == "dve":
                nc.vector.tensor_scalar(dst, src, 0.0, None, Alu.max)
            else:
                nc.gpsimd.tensor_scalar(dst, src, 0.0, None, Alu.max)

        def ew_bias_relu(dst, src, nel):
            e = pick("ts", nel, psum=True, allowed=EVICT_ENGINES)
            if e == "act":
                nc.scalar.activation(dst, src, Act.Relu, bias=b1ap)
            elif e == "dve":
                nc.vector.tensor_scalar(dst, src, b1ap, 0.0, Alu.add, Alu.max)
            else:
                nc.gpsimd.tensor_scalar(dst, src, b1ap, 0.0, Alu.add, Alu.max)

        def ew_copy(dst, src, nel, allowed=("act", "dve")):
            e = pick("ts", nel, psum=True, allowed=allowed)
            if e == "act":
                nc.scalar.copy(dst, src)
            elif e == "dve":
                nc.vector.tensor_copy(dst, src)
            else:
                nc.gpsimd.tensor_copy(dst, src)

        def ew_mul2(dst, src, nel, allowed=("act", "dve")):
            e = pick("ts", nel, allowed=allowed)
            if e == "act":
                nc.scalar.mul(dst, src, 2.0)
            elif e == "dve":
                nc.vector.tensor_scalar(dst, src, 2.0, None, Alu.mult)
            else:
                nc.gpsimd.tensor_scalar(dst, src, 2.0, None, Alu.mult)

        def ew_add(dst, a, b, nel, allowed=("dve",), kind="add"):
            e = pick(kind, nel, allowed=allowed)
            if e == "dve":
                nc.vector.tensor_tensor(dst, a, b, Alu.add)
            else:
                nc.gpsimd.tensor_tensor(dst, a, b, Alu.add)

        # ---- PE warmup (p-state ramp) on zeroed scratch ----
        if WARMUP:
            nc.gpsimd.memset(wsc[0:19, 0:256], 0.0)
            pw = p01.tile([128, UW], f32, tag="p01")
            wr = wsc[:].bitcast(f32r)
            for _ in range(WARMUP):
                nc.tensor.matmul(pw[:, 0:256], wr[0:19, 0:128],
                                 wr[0:19, 0:256], start=True, stop=True)
            # keep the BIR verifier happy: the warmup result needs a reader
            nc.vector.tensor_scalar(pw[:, 0:1], pw[:, 0:1], 0.0, None,
                                    Alu.add)

        # ---- flat unit list across phases ----
        flat = []
        for h in range(NPH):
            for u in _units(H * PHW[h]):
                flat.append((h, u))
        phase_first = {}
        for idx, (h, u) in enumerate(flat):
            phase_first.setdefault(h, idx)

        phase_state = {}
        zoff = 0
        due = []                      # (due_unit, seq, fn) staggered emissions
        due_seq = [0]

        def push_due(unit, fn):
            due.append((unit, due_seq[0], fn))
            due_seq[0] += 1
            due.sort()

        def start_phase(h):
            nonlocal zoff
            if h >= NPH:
                return
            WHp = PHW[h]
            SP = H * WHp
            zts, chunk_offs = [], []
            co = 0
            for ci, cw in enumerate(PHCHUNKS[h][:2]):
                pool_ = zs if ci == 0 else zc
                zt = pool_.tile([K0, cw], f32r, tag="z0" if ci == 0 else "z")
                nc.scalar.dma_start(zt[:], zd[:, zoff + co: zoff + co + cw])
                zts.append(zt)
                chunk_offs.append(co)
                co += cw
            y1t = y1p.tile([128, SP], f32r, tag="y1")
            mls = ML[WHp]
            cum = [0]
            for L in mls:
                cum.append(cum[-1] + L)
            phase_state[h] = {
                "zts": zts, "coffs": chunk_offs, "y1": y1t, "prog": 0,
                "mm2_ptr": 0, "p2t": [None] * len(mls), "mls": mls,
                "cum": cum, "y2": None, "blk3": None, "zco": co,
                "zbase": zoff,
            }
            zoff += SP

        def start_phase_late(h):
            ps = phase_state[h]
            co = ps["zco"]
            for cw in PHCHUNKS[h][2:]:
                zt = zc.tile([K0, cw], f32r, tag="z")
                nc.scalar.dma_start(zt[:],
                                    zd[:, ps["zbase"] + co:
                                        ps["zbase"] + co + cw])
                ps["zts"].append(zt)
                ps["coffs"].append(co)
                co += cw

        def rhs(h, o, wdt):
            ps = phase_state[h]
            ci = max(i for i, c in enumerate(ps["coffs"]) if c <= o)
            return ps["zts"][ci][:, o - ps["coffs"][ci]:
                                 o - ps["coffs"][ci] + wdt]

        st = {}

        def queue_upsample(h):
            """Queue staggered per-band upsample emissions for phase h."""
            WHp = PHW[h]
            ps = phase_state[h]
            last_phase = (h == NPH - 1)
            bands = ([(0, 5), (5, 10), (10, 14), (14, 18)]
                     if last_phase else [(0, 10), (10, 18)])
            base = phase_first[h] + len(_units(H * WHp)) - 1
            alw_ab = ("dve", "gp") if last_phase else ("gp",)
            alw_ab2 = ("dve", "gp") if last_phase else ("gp",)
            alw_c = ("dve", "gp") if last_phase else ("gp",)

            def setup():
                fxh = fxp.tile([128, (ORPB + 1) * (2 * WHp)], f32, tag="fx")
                ps["fx"] = fxh
                ps["blk3"] = ps["y2"][:].rearrange("p (j c) -> p j c",
                                                   j=RPB + 1)

            def mk_ab(bi):
                j0, j1 = bands[bi]

                def emit():
                    if "fx" not in ps:
                        setup()
                    blk3 = ps["blk3"]
                    fx3 = ps["fx"][:].rearrange("p (v c) -> p v c",
                                                v=ORPB + 1)
                    nw = j1 - j0
                    hj = blk3[:, j0:j1, :]
                    fe = fx3[:, 2 * j0:2 * j1 - 1:2, :]
                    # pass A: odd out-cols (2x values)
                    ew_mul2(fe[:, :, 1:2 * WHp:2], hj, nw * WHp,
                            allowed=alw_ab)
                    # out-col 0 (2x class); phase seam adds prev phase col
                    if h == 0:
                        ew_mul2(fe[:, :, 0:1], hj[:, :, 0:1], nw,
                                allowed=alw_ab)
                    else:
                        ew_add(fe[:, :, 0:1],
                               phase_state[h - 1]["blk3"][:, j0:j1, -1:],
                               hj[:, :, 0:1], nw, allowed=("dve", "gp"))
                    # pass B: even out-cols = adjacent in-col sums
                    ew_add(fe[:, :, 2:2 * WHp - 1:2], hj[:, :, 0:WHp - 1],
                           hj[:, :, 1:WHp], nw * (WHp - 1), allowed=alw_ab2)
                return emit

            def mk_csig(bi):
                j0, j1 = bands[bi]

                def emit():
                    fx3 = ps["fx"][:].rearrange("p (v c) -> p v c",
                                                v=ORPB + 1)
                    # pass C: odd fx rows = adjacent wout-row sums
                    vo0, vo1 = max(1, 2 * j0 - 1), 2 * j1 - 2
                    ew_add(fx3[:, vo0:vo1:2, :], fx3[:, vo0 - 1:vo1 - 1:2, :],
                           fx3[:, vo0 + 1:vo1 + 1:2, :],
                           (vo1 - vo0 + 1) // 2 * 2 * WHp, allowed=alw_c)
                    # in-place sigmoid; top boundary wout row defers to the
                    # next band (read pre-sigmoid by its pass C)
                    last_band = bi == len(bands) - 1
                    ve0 = max(2, 2 * j0 - 2)
                    ve1 = 2 * j1 - 1 if last_band else 2 * j1 - 3
                    for (v0, v1), rowsc in (((ve0, ve1), 0.5),
                                            ((vo0, vo1), 0.25)):
                        ap_ = fx3[:, v0:v1:2, :]
                        nv = (v1 - v0 + 1) // 2
                        busy["act"] += cost("act", "ts", nv * 2 * WHp, False)
                        nc.scalar.activation(ap_, ap_, Act.Sigmoid, bias=b2ap,
                                             scale=rowsc)
                    vs0 = max(1, 2 * j0 - 2)
                    vs1 = 2 * j1 - 1 if last_band else 2 * j1 - 2
                    dst = outd[:, :, vs0 - 1:vs1 - 1,
                               2 * PHOFF[h]: 2 * (PHOFF[h] + WHp)]
                    sfx = ps["fx"][:, vs0 * 2 * WHp: vs1 * 2 * WHp] \
                        .rearrange("p (v c) -> p v c", c=2 * WHp)
                    nc.sync.dma_start(dst, sfx)
                return emit

            for k in range(len(bands)):
                push_due(base + DELAY + 2 * k, mk_ab(k))
                push_due(base + DELAY + 2 * k + 3, mk_csig(k))

        def emit_mm2_ready(h, final=False):
            ps = phase_state[h]
            WHp = PHW[h]
            LB = RPB * WHp
            mls, cum = ps["mls"], ps["cum"]
            nm = len(mls)
            prog = ps["prog"] - (0 if final else M2SLACK * UW)
            while ps["mm2_ptr"] < BLK * nm:
                b, m = divmod(ps["mm2_ptr"], nm)
                if LB * b + cum[m + 1] > prog:
                    break
                if ps["p2t"][m] is None:
                    p2t = p2.tile([128, 512], f32, tag="p2")
                    ps["p2t"][m] = p2t
                L = mls[m]
                nc.tensor.matmul(ps["p2t"][m][:, 0:L],
                                 w2v[:, 128 * b:128 * (b + 1)],
                                 ps["y1"][:, LB * b + cum[m]:
                                          LB * b + cum[m] + L],
                                 start=(b == 0), stop=(b == BLK - 1),
                                 skip_group_check=True)
                ps["mm2_ptr"] += 1
                if b == BLK - 1:
                    if ps["y2"] is None:
                        y2t = y2p.tile([128, 18 * WHp], f32, tag="y2")
                        ps["y2"] = y2t
                    ew_copy(ps["y2"][:, WHp + cum[m]: WHp + cum[m] + L],
                            ps["p2t"][m][:, 0:L], L)
                    if m == 0:
                        # block-0 halo = duplicate of its row 0
                        nc.sync.dma_start(ps["y2"][0:16, 0:WHp],
                                          ps["y2"][0:16, WHp:2 * WHp])
                    if m == nm - 1:
                        # halo for blocks 1..7 = previous block's last row
                        nc.sync.dma_start(
                            ps["y2"][16:128, 0:WHp],
                            ps["y2"][0:112, RPB * WHp: (RPB + 1) * WHp])
                        queue_upsample(h)

        def s_mm0(i):
            h, (off, w) = flat[i]
            if h not in phase_state:
                start_phase(h)
            if phase_first[h] == i:
                start_phase_late(h)
            if i + PREFETCH < len(flat):
                hn = flat[i + PREFETCH][0]
                if hn not in phase_state:
                    start_phase(hn)
            p0t = p01.tile([128, UW], f32, tag="p01")
            nc.tensor.matmul(p0t[:, 0:w], a0r, rhs(h, off, w),
                             start=True, stop=True)
            y0t = y0p.tile([128, UW], f32r, tag="y0")
            ew_relu(y0t[:, 0:w], p0t[:, 0:w], w)
            st[i] = y0t

        def s_mm1(i):
            h, (off, w) = flat[i]
            y0t = st.pop(i)
            p1t = p01.tile([128, UW], f32, tag="p01")
            nc.tensor.matmul(p1t[:, 0:w], w1r, y0t[:, 0:w],
                             start=True, stop=True)
            ps = phase_state[h]
            ew_bias_relu(ps["y1"][:, off:off + w], p1t[:, 0:w], w)
            ps["prog"] = off + w
            if not ISOLATE:
                emit_mm2_ready(h, final=(off + w == H * PHW[h]))

        for i in range(len(flat) + LAG + 64):
            if i == 1:
                nc.sync.dma_start(wp[:], wpd)
                nc.sync.dma_start(w2v[:], w2vd)
            if i < len(flat):
                s_mm0(i)
            if 0 <= i - LAG < len(flat):
                s_mm1(i - LAG)
            while due and due[0][0] <= i - LAG:
                due.pop(0)[2]()
            if not due and i - LAG >= len(flat):
                break

    nc.compile()
    return nc


def _host_prep(mask_feats, mask_head_params, locations, im_inds, fpn_levels,
               sizes_of_interest):
    mask_feats = np.asarray(mask_feats, dtype=np.float32)
    params = np.asarray(mask_head_params, dtype=np.float32)
    locations = np.asarray(locations, dtype=np.float32)
    im_inds = np.asarray(im_inds).astype(np.int64)
    fpn_levels = np.asarray(fpn_levels).astype(np.int64)
    soi_tab = np.asarray(sizes_of_interest, dtype=np.float32)

    w0 = params[:, 0:80].reshape(N_INST, CH, CIN + 2)
    w1 = params[:, 80:144].reshape(N_INST, CH, CH)
    w2 = params[:, 144:152].reshape(N_INST, 1, CH)
    b0 = params[:, 152:160]
    b1 = params[:, 160:168]
    b2 = params[:, 168:169]

    soi = soi_tab[fpn_levels]                                    # (128,)
    alpha = -w0[:, :, 0] / soi[:, None]
    beta = -w0[:, :, 1] / soi[:, None]
    c0 = b0 + (w0[:, :, 0] * locations[:, 0:1]
               + w0[:, :, 1] * locations[:, 1:2]) / soi[:, None]
    wfeat = w0[:, :, 2:]                                         # (128, 8, 8)

    stride = 8
    xs = np.arange(W, dtype=np.float32) * stride + stride // 2
    ys = np.arange(H, dtype=np.float32) * stride + stride // 2
    locs_x = np.tile(xs, H)
    locs_y = np.repeat(ys, W)
    z = np.concatenate([locs_x[None], locs_y[None],
                        np.ones((1, HW), np.float32),
                        mask_feats.reshape(N_IMG * CIN, HW)], axis=0)
    z3 = z.reshape(K0, H, W)
    z = np.concatenate(
        [z3[:, :, PHOFF[q]:PHOFF[q] + PHW[q]].reshape(K0, H * PHW[q])
         for q in range(NPH)], axis=1)
    z = np.ascontiguousarray(z, dtype=np.float32)

    in_maps = []
    for c in range(N_CORES):
        a0 = np.zeros((K0, 128), np.float32)
        wpack = np.zeros((128, 130), np.float32)
        w2vv = np.zeros((BLK, 128, 128), np.float32)
        for i in range(IPC):
            gi = IPC * c + i
            for o in range(CH):
                m = CH * i + o
                a0[0, m] = alpha[gi, o]
                a0[1, m] = beta[gi, o]
                a0[2, m] = c0[gi, o]
                base = 3 + CIN * int(im_inds[gi])
                a0[base:base + CIN, m] = wfeat[gi, o, :]
                wpack[CH * i:CH * i + CH, m] = w1[gi, o, :]
                wpack[m, 128] = b1[gi, o]
            for b in range(BLK):
                w2vv[b, CH * i:CH * i + CH, IPC * b + i] = w2[gi, 0, :]
                wpack[IPC * b + i, 129] = b2[gi, 0]
        in_maps.append({
            "z_in": z,
            "a0t_in": np.ascontiguousarray(a0),
            "wpack_in": np.ascontiguousarray(wpack),
            "w2v_in": np.ascontiguousarray(
                w2vv.transpose(1, 0, 2).reshape(128, BLK * 128)),
        })
    return in_maps


def kernel(mask_feats, mask_head_params, locations, im_inds, fpn_levels,
           sizes_of_interest, mask_feat_stride):
    global LAST_EXEC_TIME_NS
    assert int(mask_feat_stride) == 8, "kernel hardcodes mask_feat_stride=8"

    from concourse.bass_utils import run_bass_kernel_spmd

    in_maps = _host_prep(mask_feats, mask_head_params, locations, im_inds,
                         fpn_levels, sizes_of_interest)

    if "nc" not in _CACHE:
        _CACHE["nc"] = _build_program()
    nc = _CACHE["nc"]

    trace = bool(os.environ.get("BASS_TRACE"))
    res = run_bass_kernel_spmd(nc, in_maps, list(range(N_CORES)), trace=trace)
    LAST_EXEC_TIME_NS = res.exec_time_ns

    out = np.empty((N_INST, 1, OH, OW), np.float32)
    for c in range(N_CORES):
        o = np.asarray(res.results[c]["out"])          # (BLK, IPC, ORPB, OW)
        out[IPC * c:IPC * (c + 1), 0] = \
            o.transpose(1, 0, 2, 3).reshape(IPC, OH, OW)
    return out
